# revision 2
# baseline (speedup 1.0000x reference)
"""Trainium2 Bass kernel for nn_ClassificationHead: LayerNorm -> Linear(1024,256) -> GELU -> Linear(256,2).

Data-parallel over 8 NeuronCores: each core processes 8192 rows of the
65536-row batch; the tiny weights are replicated.

Per-core pipeline (per 128-row tile):
  1. SWDGE cast-DMA loads the fp32 rows from HBM as bf16 into a natural-layout
     SBUF tile [128 rows, 1024+128 cols].
  2. DVE bn_stats/bn_aggr computes per-row mean/var; ACT computes
     rhat = sqrt(var+eps); DVE writes (-mu, rhat) into two spare columns.
  3. One HWDGE xbar-transpose DMA produces the K-major tile [128, 9, 128];
     the stats columns become two extra contraction rows.
  4. TensorE: 8 accumulating matmuls (x @ W1') + a rank-2 matmul with
     rhs=[s1; c1] that adds (-mu*s1 + rhat*c1) — so after scaling by
     g = 1/rhat (fused into the GELU's per-partition scale) the PSUM holds
     exactly gelu-input = LN(x) @ W1' + b1' .
  5. ACT evaluates exact GELU with per-partition scale g -> bf16 h tile.
  6. TensorE transposes h (via identity), ACT evacuates PSUM->SBUF bf16,
     TensorE computes h @ W2; DVE adds b2 into a staging tile.
  7. One DMA writes the [8192, 2] fp32 result back.

Weight folding done on host (tiny, O(1MB)): W1' = ln_w[:,None]*W1,
s1 = colsum(W1'), c1 = ln_b@W1 + b1.
"""
import sys

sys.path.insert(0, "/opt/trn_rl_repo")
sys.path.insert(0, "/root/.axon_site")

import numpy as np
import ml_dtypes

N_CORES = 8
BATCH = 65536
D = 1024
H = 256
OUT = 2
RPC = BATCH // N_CORES  # rows per core
NT = RPC // 128         # 128-row tiles per core
KC = D // 128           # contraction chunks
EPS = 1e-5

_cache = {}


def _bf16(a):
    return np.asarray(a, dtype=ml_dtypes.bfloat16)


def _build():
    import concourse.bacc as bacc
    import concourse.mybir as mybir
    from concourse import tile

    f32 = mybir.dt.float32
    bf16 = mybir.dt.bfloat16
    AF = mybir.ActivationFunctionType
    ALU = mybir.AluOpType

    nc = bacc.Bacc(None, target_bir_lowering=False, debug=False)

    x_in = nc.dram_tensor("x", [RPC, D], f32, kind="ExternalInput")
    w1_in = nc.dram_tensor("w1b", [128, KC, H], bf16, kind="ExternalInput")
    sc_in = nc.dram_tensor("sc", [2, H], bf16, kind="ExternalInput")
    w2_in = nc.dram_tensor("w2b", [128, 2, OUT], bf16, kind="ExternalInput")
    b2_in = nc.dram_tensor("b2r", [128, OUT], f32, kind="ExternalInput")
    id_in = nc.dram_tensor("ident", [128, 128], bf16, kind="ExternalInput")
    y_out = nc.dram_tensor("y", [RPC, OUT], f32, kind="ExternalOutput")

    x_t = x_in.rearrange("(t p) d -> t p d", p=128)
    y_t = y_out.rearrange("(t p) c -> p t c", p=128)

    with tile.TileContext(nc) as tc:
        with (
            tc.tile_pool(name="wpool", bufs=1) as wp,
            tc.tile_pool(name="natp", bufs=3) as natp,
            tc.tile_pool(name="xtp", bufs=3) as xtp,
            tc.tile_pool(name="statp", bufs=4) as statp,
            tc.tile_pool(name="hbp", bufs=3) as hbp,
            tc.tile_pool(name="htp", bufs=3) as htp,
            tc.tile_pool(name="outp", bufs=1) as outp,
            tc.tile_pool(name="pszp", bufs=3, space="PSUM") as pszp,
            tc.tile_pool(name="pstp", bufs=2, space="PSUM") as pstp,
            tc.tile_pool(name="psop", bufs=2, space="PSUM") as psop,
        ):
            w1sb = wp.tile([128, KC, H], bf16)
            nc.sync.dma_start(w1sb[:], w1_in[:])
            scsb = wp.tile([2, H], bf16)
            nc.sync.dma_start(scsb[:], sc_in[:])
            w2sb = wp.tile([128, 2, OUT], bf16)
            nc.sync.dma_start(w2sb[:], w2_in[:])
            b2sb = wp.tile([128, OUT], f32)
            nc.sync.dma_start(b2sb[:], b2_in[:])
            idsb = wp.tile([128, 128], bf16)
            nc.sync.dma_start(idsb[:], id_in[:])

            outsb = outp.tile([128, NT, OUT], f32)

            for t in range(NT):
                natb = natp.tile([128, D + 128], bf16)
                nc.gpsimd.dma_start(natb[:, 0:D], x_t[t])

                bst = statp.tile([128, 12], f32)
                nc.vector.bn_stats(bst[:, 0:6], natb[:, 0:512])
                nc.vector.bn_stats(bst[:, 6:12], natb[:, 512:1024])
                agg = statp.tile([128, 2], f32)
                nc.vector.bn_aggr(agg[:], bst[:])

                vh = statp.tile([128, 1], f32)
                nc.vector.tensor_scalar_add(vh[:], agg[:, 1:2], EPS)
                rt = statp.tile([128, 1], f32)
                nc.scalar.activation(rt[:], vh[:], AF.Sqrt, bias=0.0, scale=1.0)
                g = statp.tile([128, 1], f32)
                nc.vector.reciprocal(g[:], rt[:])
                # stats columns: col D = -mu, col D+1 = rhat (both bf16)
                nc.vector.tensor_scalar_mul(natb[:, D : D + 1], agg[:, 0:1], -1.0)
                nc.vector.tensor_copy(natb[:, D + 1 : D + 2], rt[:])

                xt = xtp.tile([128, KC + 1, 128], bf16)
                nc.sync.dma_start(xt[:], natb[:], transpose=True)

                psz = pszp.tile([128, H], f32)
                for k in range(KC):
                    nc.tensor.matmul(
                        psz[:], xt[:, k, :], w1sb[:, k, :], start=(k == 0), stop=False
                    )
                nc.tensor.matmul(psz[:], xt[0:2, KC, :], scsb[:], start=False, stop=True)

                hb = hbp.tile([128, H], bf16)
                nc.scalar.activation(hb[:], psz[:], AF.Gelu, bias=0.0, scale=g[:, 0:1])

                pst = pstp.tile([128, H], bf16)
                nc.tensor.transpose(pst[:, 0:128], hb[:, 0:128], idsb[:])
                nc.tensor.transpose(pst[:, 128:256], hb[:, 128:256], idsb[:])
                ht = htp.tile([128, 2, 128], bf16)
                nc.scalar.copy(ht[:], pst[:])

                pso = psop.tile([128, OUT], f32)
                nc.tensor.matmul(pso[:], ht[:, 0, :], w2sb[:, 0, :], start=True, stop=False)
                nc.tensor.matmul(pso[:], ht[:, 1, :], w2sb[:, 1, :], start=False, stop=True)

                nc.vector.tensor_add(outsb[:, t, :], pso[:], b2sb[:])

            nc.sync.dma_start(y_t[:], outsb[:])

    nc.finalize()
    return nc


def _get_nc():
    if "nc" not in _cache:
        _cache["nc"] = _build()
    return _cache["nc"]


def kernel(embedding, ln_w, ln_b, W1, b1, W2, b2):
    from concourse.bass_utils import run_bass_kernel_spmd

    embedding = np.asarray(embedding, dtype=np.float32)
    ln_w = np.asarray(ln_w, dtype=np.float32)
    ln_b = np.asarray(ln_b, dtype=np.float32)
    W1 = np.asarray(W1, dtype=np.float32)
    b1 = np.asarray(b1, dtype=np.float32)
    W2 = np.asarray(W2, dtype=np.float32)
    b2 = np.asarray(b2, dtype=np.float32)

    # Host-side weight folding (tiny tensors only).
    W1p = ln_w[:, None] * W1                      # [1024, 256]
    s1 = W1p.sum(axis=0)                          # [256]
    c1 = ln_b @ W1 + b1                           # [256]
    w1b = _bf16(W1p.reshape(KC, 128, H).transpose(1, 0, 2))
    sc = _bf16(np.stack([s1, c1]))
    w2b = _bf16(W2.reshape(2, 128, OUT).transpose(1, 0, 2))
    b2r = np.broadcast_to(b2, (128, OUT)).astype(np.float32)
    ident = _bf16(np.eye(128, dtype=np.float32))

    nc = _get_nc()
    shards = embedding.reshape(N_CORES, RPC, D)
    in_maps = [
        {
            "x": shards[c],
            "w1b": w1b,
            "sc": sc,
            "w2b": w2b,
            "b2r": b2r,
            "ident": ident,
        }
        for c in range(N_CORES)
    ]
    res = run_bass_kernel_spmd(nc, in_maps, core_ids=list(range(N_CORES)))
    out = np.concatenate([res.results[c]["y"] for c in range(N_CORES)], axis=0)
    return out.astype(np.float32)
